# revision 52
# baseline (speedup 1.0000x reference)
"""Trainium2 Bass kernel for nn_NodeDetector (masked-node GATv2 ensemble).

Algorithm: the reference vmaps a full 2-layer GATv2 over 256 "masked node"
variants, but variant v differs from the shared base computation in exactly
one input row (row v).  We compute the base graph once and apply sparse
incremental updates per variant:

  phase 0  dense projections -> XL/XR (base rows) and XLs/XRs (masked rows)
  phase 1  base GAT layer 1: per-dst softmax sums (num1/den1) + g1_base
  (a)      per variant v: "light" g1 updates at out-neighbors d of v
           (only edges v->d changed: closed-form num/den delta)
  (b)      per variant v: full recompute of g1 at node v
  (d)      layer 2 at dst v only: gather xl2 of in-neighbors (base / self
           rows); in-neighbors that are also out-neighbors ("rare") get a
           separate 128-row mini edge-stage whose contribution accumulates
           into the same PSUM as the main combine, so the xl2 gather does
           not wait for the (a) phase.

Attention softmaxes skip the per-dst max subtraction (mathematically
identical; logits are O(10) so fp32 exp is safe).  All gathers use
host-built index tables (edge_index is host data) via gpsimd indirect DMA.
Work is sharded 32 variants per core across 8 cores; phases 0/1 are
replicated per core.  No collectives (cross-core launch skew lands in the
max-core exec time).

Perf notes:
  - gpsimd descriptor generation costs ~8.6ns per gathered row regardless
    of row bytes, so everything is organized to minimize gathered ROWS:
    edge slots are degree-aware bin-packed; the (a) phase uses
    variant-pure partitions so per-variant XL rows are a 128-row indirect
    gather + free partition-broadcast instead of a 640-row gather.
  - tables/edge math in bf16 (fp16 breaks on this stack); softmax sums,
    the (a)-phase delta math and final head math stay f32.
  - base-XL is stored to its own table (XLbase) right after its matmul so
    the first big gather starts ~18us in, before the XR chain finishes.
  - all constants/index tables ship in 4 packed DMAs (dma_start issue
    costs ~600ns each on the sync queue).
  - p1 lrelu's scale runs on the scalar engine: the equivalent DVE op
    suffers ~10x SBUF contention while gather DMAs write concurrently.
"""

import numpy as np

import concourse.bass as bass
import concourse.mybir as mybir
import concourse.tile as tile
from concourse import bacc
from concourse.bass_utils import run_bass_kernel_spmd
from concourse.masks import make_identity

F32 = mybir.dt.float32
BF16 = mybir.dt.bfloat16
I32 = mybir.dt.int32
I16 = mybir.dt.int16
AF = mybir.ActivationFunctionType
OP = mybir.AluOpType
AX = mybir.AxisListType

N = 256          # nodes / variants
F = 128          # NUM_HEAD * C2
C2 = 64
NH = 2
NCORES = 8
VPC = N // NCORES   # variants per core = 32
NEG = 0.2           # leaky relu slope


# --------------------------------------------------------------------------
# Host-side table construction
# --------------------------------------------------------------------------

def _wrap16(flat):
    """int16 idx layout for dma_gather: value for flat position i lives
    at [i % 16, i // 16], tiled to 128 partitions."""
    flat = np.asarray(flat)
    num = flat.shape[0]
    A = np.zeros((16, num // 16), np.int16)
    A[np.arange(num) % 16, np.arange(num) // 16] = flat.astype(np.int16)
    return np.ascontiguousarray(np.tile(A, (8, 1)))


def _wrapPK(idx_pk):
    """[128, K] logical idx (out[p, k] = tab[idx_pk[p,k]]) -> wrapped."""
    return _wrap16(idx_pk.T.reshape(-1))


def _min_slots(degs, nrows):
    S = 1
    while sum(-(-d // S) for d in degs) > nrows:
        S += 1
    return S


def _binpack(keys, sizes, nrows, S):
    """Rows of (key, [chunk indices]), <= S items each, padded to nrows."""
    rows = []
    for key, sz in zip(keys, sizes):
        for i in range(0, sz, S):
            rows.append((key, list(range(i, min(i + S, sz)))))
    assert len(rows) <= nrows
    rows += [None] * (nrows - len(rows))
    return rows


def _build_tables(edge_index):
    src = edge_index[0].astype(np.int64)
    dst = edge_index[1].astype(np.int64)
    E = src.shape[0]

    in_edges = [[] for _ in range(N)]
    for e in range(E):
        in_edges[dst[e]].append(e)

    # p1: two halves of 128 dst, 512 rows each, degree-aware slot packing
    S1 = max(_min_slots([len(in_edges[d]) for d in range(128 * h,
                                                         128 * (h + 1))], 512)
             for h in range(2))
    # b/d: per-core 32 dst over 128 rows; one global slot count
    SB = max(_min_slots([len(in_edges[v]) for v in range(VPC * c,
                                                         VPC * (c + 1))], 128)
             for c in range(NCORES))

    out_by_src = [[] for _ in range(N)]
    for e in range(E):
        if dst[e] != src[e]:
            out_by_src[src[e]].append(int(dst[e]))
    light = []
    for v in range(N):
        cnt = {}
        for d in out_by_src[v]:
            cnt[d] = cnt.get(d, 0) + 1
        light.append(sorted(cnt.items()))
    # (a) phase: light pairs bin-packed into variant-pure partition rows
    KA = max(_min_slots([len(light[v]) for v in range(VPC * c,
                                                      VPC * (c + 1))], 128)
             for c in range(NCORES))

    shared = {}
    IDX1W = np.zeros((2, 128, 4 * S1 * 8), np.int16)
    MSK1 = np.zeros((2, 128, 4 * S1), np.float32)
    OHXR1 = np.zeros((2, 4, 128, 128), np.float32)   # [h][t][node, p]
    CMB1 = np.zeros((2, 128, 4, 128), np.float32)
    for h in range(2):
        dsts = list(range(128 * h, 128 * (h + 1)))
        rows = _binpack(dsts, [len(in_edges[d]) for d in dsts], 512, S1)
        pk = np.zeros((128, 4 * S1), np.int64)
        for r, ent in enumerate(rows):
            t, p = divmod(r, 128)
            if ent is None:
                continue
            d, ch = ent
            OHXR1[h, t, d - 128 * h, p] = 1.0
            CMB1[h, p, t, d - 128 * h] = 1.0
            for si, k in enumerate(ch):
                pk[p, t * S1 + si] = src[in_edges[d][k]]
                MSK1[h, p, t * S1 + si] = 1.0
        IDX1W[h] = _wrapPK(pk)
    shared["IDX1W"] = IDX1W
    shared["MSK1"] = MSK1
    shared["OHXR1"] = OHXR1
    shared["CMB1"] = CMB1

    percore = []
    for c in range(NCORES):
        t = {}
        V = list(range(c * VPC, (c + 1) * VPC))

        # ---- (a) layout: variant-pure rows of light pairs ----
        rows_a = _binpack(list(range(VPC)),
                          [len(light[V[vi]]) for vi in range(VPC)], 128, KA)
        IDX_A_T1 = np.zeros((128, KA), np.int64)
        C_A = np.zeros((128, KA), np.float32)
        OHXLV = np.zeros((2, 128, 128), np.float32)    # [chunk][node, p]
        OHXLSV = np.zeros((2, 128, 128), np.float32)
        arow = {}                       # (vi, light slot) -> G1L row p*KA+k
        for p, ent in enumerate(rows_a):
            if ent is None:
                continue
            vi, ch = ent
            v = V[vi]
            OHXLV[v // 128, v % 128, p] = 1.0
            OHXLSV[v // 128, v % 128, p] = 1.0   # XLs chunks come after XL
            for k, li in enumerate(ch):
                d, cc = light[v][li]
                IDX_A_T1[p, k] = d
                C_A[p, k] = float(cc)
                arow[(vi, li)] = p * KA + k
        t["IDX_A_T1W"] = _wrapPK(IDX_A_T1)
        t["C_A"] = C_A
        t["OHXLV"] = OHXLV
        t["OHXLSV"] = OHXLSV

        # ---- b/d rows ----
        rows = _binpack(V, [len(in_edges[v]) for v in V], 128, SB)
        IDXB = np.zeros((128, SB), np.int64)
        IDXD2 = np.zeros((128, SB), np.int64)
        MSKBD = np.zeros((128, SB), np.float32)
        MSKD = np.zeros((128, SB), np.float32)   # rare slots zeroed
        CMBBD = np.zeros((128, VPC), np.float32)
        OHB = np.zeros((2, 128, 128), np.float32)      # [chunk][node, r]
        OHD2 = np.zeros((32, 128), np.float32)         # [vi, r]
        OHRARE2 = np.zeros((32, 128), np.float32)      # [vi, rare row]
        OHRARE_K = np.zeros((KA, 128, 128), np.float32)  # [k][p, rare row]
        OH_RARE = np.zeros((128, VPC), np.float32)
        C_RARE = np.zeros((128, 1), np.float32)
        C_SELF = np.zeros((VPC, 1), np.float32)
        rare_map = {}
        for r, ent in enumerate(rows):
            if ent is None:
                continue
            v, ch = ent
            vi = v - c * VPC
            CMBBD[r, vi] = 1.0
            OHB[v // 128, v % 128, r] = 1.0   # XRs chunk of node v
            OHD2[vi, r] = 1.0                 # XR2S row
            lpos = {d: i for i, (d, _) in enumerate(light[v])}
            for si, k in enumerate(ch):
                sn = int(src[in_edges[v][k]])
                MSKBD[r, si] = 1.0
                IDXB[r, si] = 256 + v if sn == v else sn
                if sn == v:
                    IDXD2[r, si] = 0      # self handled by the mini stage
                    C_SELF[vi, 0] += 1.0
                elif sn in lpos:
                    # rare: handled by the mini edge-stage, slot masked
                    key = (vi, sn)
                    if key not in rare_map:
                        rs = len(rare_map)
                        assert rs < 128, "rare-row overflow"
                        rare_map[key] = rs
                        g1l_row = arow[(vi, lpos[sn])]
                        OHRARE_K[g1l_row % KA, g1l_row // KA, rs] = 1.0
                        OHRARE2[vi, rs] = 1.0
                        OH_RARE[rs, vi] = 1.0
                    C_RARE[rare_map[key], 0] += 1.0
                    IDXD2[r, si] = 0
                else:
                    IDXD2[r, si] = sn
                    MSKD[r, si] = 1.0
        t["IDX_BW"] = _wrapPK(IDXB)
        t["IDX_DW"] = _wrapPK(IDXD2)
        t["MSKBD"] = MSKBD
        t["MSKD"] = MSKD
        t["CMBBD"] = CMBBD
        t["OH_RARE"] = OH_RARE
        t["C_RARE"] = C_RARE
        t["C_SELF"] = C_SELF
        t["OHB"] = OHB
        t["OHD2"] = OHD2
        t["OHRARE2"] = OHRARE2
        t["OHRARE_K"] = OHRARE_K
        percore.append(t)

    dims = dict(S1=S1, SB=SB, KA=KA)
    return shared, percore, dims


# --------------------------------------------------------------------------
# Packed-input layouts (single source of truth for device + host)
# --------------------------------------------------------------------------

def _pack_layout(dims):
    S1, SB, KA = dims["S1"], dims["SB"], dims["KA"]
    pf = [("node_proj", 64, 128), ("emb_proj", 64, 128),
          ("conv_w0", 128, 128), ("conv_w1", 128, 128), ("conv_b", 128, 1),
          ("WLn", 128, 128), ("WLm", 128, 128), ("WRn", 128, 128),
          ("WRm", 128, 128), ("bLn", 128, 1), ("bLm", 128, 1),
          ("bRn", 128, 1), ("bRm", 128, 1), ("g2_wl", 64, 128),
          ("g2_wr", 64, 128), ("rec_w", 64, 64), ("rec_b", 64, 1),
          ("att1", 128, 128), ("att2", 128, 128), ("g1bias", 128, 64),
          ("g2bias", 128, 64), ("blr", 128, 128),
          ("CMB1", 128, 8 * 128), ("CMBBD", 128, VPC),
          ("OH_RARE", 128, VPC), ("C_RARE", 128, 1),
          ("MSK1", 128, 8 * S1), ("MSKBD", 128, SB), ("MSKD", 128, SB),
          ("C_A", 128, KA), ("C_SELF", 128, 1),
          ("OHRARE_K", 128, KA * 128)]
    pi16 = [("IDX1W0", 128, 4 * S1 * 8), ("IDX1W1", 128, 4 * S1 * 8),
            ("IDX_BW", 128, SB * 8), ("IDX_DW", 128, SB * 8),
            ("IDX_A_T1W", 128, KA * 8)]
    # bf16 one-hot row-select matrices (PE replaces small indirect gathers)
    pbf = [("OHXR1", 128, 8 * 128), ("OHB", 128, 2 * 128),
           ("OHXLV", 128, 2 * 128), ("OHXLSV", 128, 2 * 128),
           ("OHD2", 128, 128), ("OHRARE2", 128, 128)]

    def offsets(lst):
        off, o = {}, 0
        for nm, rows, cols in lst:
            off[nm] = (o, rows, cols)
            o += cols
        return off, o

    return offsets(pf), offsets(pi16), offsets(pbf)


# --------------------------------------------------------------------------
# Device program
# --------------------------------------------------------------------------

def _build_program(dims):
    S1, SB, KA = dims["S1"], dims["SB"], dims["KA"]
    (off_f, nf), (off_i16, ni16), (off_bf, nbf) = _pack_layout(dims)

    nc = bacc.Bacc("TRN2", target_bir_lowering=False, debug=False)

    D = {}
    D["packf"] = nc.dram_tensor("packf", [128, nf], F32,
                                kind="ExternalInput")
    D["packi16"] = nc.dram_tensor("packi16", [128, ni16], I16,
                                  kind="ExternalInput")
    D["xE"] = nc.dram_tensor("xE", [64, 512], F32, kind="ExternalInput")
    D["packbf"] = nc.dram_tensor("packbf", [128, nbf], BF16,
                                 kind="ExternalInput")

    D["out"] = nc.dram_tensor("out", [VPC, 64], F32, kind="ExternalOutput")
    D["XLbase"] = nc.dram_tensor("XLbase", [N, F], BF16)
    D["XLcat"] = nc.dram_tensor("XLcat", [2 * N, F], BF16)
    D["XRcat"] = nc.dram_tensor("XRcat", [2 * N, F], BF16)
    D["T1"] = nc.dram_tensor("T1", [N, 192], F32)   # row 768B; 130 used
    D["T2"] = nc.dram_tensor("T2", [N, F], BF16)

    with tile.TileContext(nc) as tc:
        _trace(nc, tc, D, dims, (off_f, off_i16, off_bf))
    nc.compile()
    return nc


def _trace(nc, tc, D, dims, offs):
    S1, SB, KA = dims["S1"], dims["SB"], dims["KA"]
    off_f, off_i16, off_bf = offs
    import contextlib
    ctx = contextlib.ExitStack()
    with ctx:
        consts = ctx.enter_context(tc.tile_pool(name="consts", bufs=1))
        small = ctx.enter_context(tc.tile_pool(name="small", bufs=1))
        big = ctx.enter_context(tc.tile_pool(name="big", bufs=1))
        psum = ctx.enter_context(tc.tile_pool(name="psum", bufs=4,
                                              space="PSUM"))
        psum_acc = ctx.enter_context(tc.tile_pool(name="psacc", bufs=2,
                                                  space="PSUM"))

        dma = nc.sync.dma_start
        sdma = nc.scalar.dma_start   # parallel issue path for hot stores

        def dgather(out_ap, in_ap, idx_ap, num, elem):
            nc.gpsimd.dma_gather(out_ap=out_ap, in_ap=in_ap, idxs_ap=idx_ap,
                                 num_idxs=num, num_idxs_reg=num,
                                 elem_size=elem, single_packet=False)
        tt = nc.vector.tensor_tensor
        red = nc.vector.tensor_reduce
        act = nc.scalar.activation
        gather = nc.gpsimd.indirect_dma_start
        IOA = bass.IndirectOffsetOnAxis

        # ---------------- constants (packed DMAs) ----------------
        ident = consts.tile([128, 128], F32, tag="ident")
        make_identity(nc, ident[:])

        xE = consts.tile([64, 512], F32, tag="xE")   # [x^T | E^T]
        dma(out=xE[:], in_=D["xE"][:])
        packf = consts.tile([128, D["packf"].shape[1]], F32, tag="packf")
        dma(out=packf[:], in_=D["packf"][:])
        packi16 = consts.tile([128, D["packi16"].shape[1]], I16,
                              tag="packi16")
        dma(out=packi16[:], in_=D["packi16"][:])
        packbf = consts.tile([128, D["packbf"].shape[1]], BF16,
                             tag="packbf")
        dma(out=packbf[:], in_=D["packbf"][:])

        def cv(name):
            o, rows, cols = off_f[name]
            return packf[:rows, o:o + cols]

        def iv16(name):
            o, rows, cols = off_i16[name]
            return packi16[:rows, o:o + cols]

        def bv(name):
            o, rows, cols = off_bf[name]
            return packbf[:rows, o:o + cols]

        def row_select(lhsTs, rhs_chunks, out_sb, tag):
            """out_sb[r, :] = sum_c lhsTs[c]^T @ rhs_chunks[c] -- PE one-hot
            row select from SBUF node-major chunks; copy via scalar ACT."""
            M, Nf = out_sb.shape[0], out_sb.shape[1]
            ps = psum.tile([128, 128], F32, tag="ps")
            for i, (lh, rh) in enumerate(zip(lhsTs, rhs_chunks)):
                nc.tensor.matmul(ps[:M, :Nf], lh, rh,
                                 start=(i == 0),
                                 stop=(i == len(lhsTs) - 1))
            act(out=out_sb, in_=ps[:M, :Nf], func=AF.Identity)

        # bf16 copies of the edge-pipeline constants
        def bfcast(name):
            o, rows, cols = off_f[name]
            t_ = consts.tile([rows, cols], BF16, tag="bf_" + name)
            nc.vector.tensor_copy(out=t_[:], in_=packf[:rows, o:o + cols])
            return t_[:]

        # ---------------- helpers ----------------
        def ts_mul(out, in0, s):
            nc.vector.tensor_scalar_mul(out=out, in0=in0, scalar1=s)

        def lrelu(flat_ap, nfree, tag, dt=F32, on_act=False):
            t_ = big.tile([128, nfree], dt, tag="lr_" + tag)
            ta = t_[:flat_ap.shape[0], :]
            if on_act:
                act(out=ta, in_=flat_ap, func=AF.Identity, scale=NEG)
            else:
                ts_mul(ta, flat_ap, NEG)
            tt(out=flat_ap, in0=flat_ap, in1=ta, op=OP.max)

        def elu_inplace(x_ap, scratch_pool, nfree, tag):
            xpos = scratch_pool.tile([128, nfree], F32, tag=tag + "_xp")
            nrow = x_ap.shape[0]
            xp = xpos[:nrow, :]
            nc.vector.tensor_scalar_max(out=xp, in0=x_ap, scalar1=0.0)
            nc.vector.tensor_scalar_min(out=x_ap, in0=x_ap, scalar1=0.0)
            act(out=x_ap, in_=x_ap, func=AF.Exp)
            nc.vector.tensor_scalar_add(out=x_ap, in0=x_ap, scalar1=-1.0)
            nc.vector.tensor_add(out=x_ap, in0=x_ap, in1=xp)
            return x_ap

        def head_mean_bias_elu(nd_ap, nrow, bias_rep, tag):
            """nd_ap [nrow, F+NH] = (num|den) -> elu(mean_h(num/den)+bias)."""
            rec = small.tile([128, NH], F32, tag=tag + "_rec")
            nc.vector.reciprocal(out=rec[:nrow, :], in_=nd_ap[:, F:F + NH])
            r0 = small.tile([128, C2], F32, tag=tag + "_r0")
            r1 = small.tile([128, C2], F32, tag=tag + "_r1")
            ts_mul(r0[:nrow, :], nd_ap[:, 0:C2], rec[:nrow, 0:1])
            ts_mul(r1[:nrow, :], nd_ap[:, C2:F], rec[:nrow, 1:2])
            tt(out=r0[:nrow, :], in0=r0[:nrow, :], in1=r1[:nrow, :], op=OP.add)
            ts_mul(r0[:nrow, :], r0[:nrow, :], 0.5)
            tt(out=r0[:nrow, :], in0=r0[:nrow, :], in1=bias_rep[:nrow, :],
               op=OP.add)
            return elu_inplace(r0[:nrow, :], small, C2, tag)

        # ---------------- phase 0 ----------------
        def mm_to_sbuf(lhsT, rhs, M, Nf, tag, bias=None, func=AF.Identity,
                       extra=None):
            out_tile = small.tile([M, Nf], F32, tag=tag)
            ps = psum.tile([128, 256], F32, tag="ps")
            nc.tensor.matmul(ps[:M, :Nf], lhsT, rhs, start=True,
                             stop=extra is None)
            if extra is not None:
                nc.tensor.matmul(ps[:M, :Nf], extra[0], extra[1],
                                 start=False, stop=True)
            if bias is None:
                act(out=out_tile[:], in_=ps[:M, :Nf], func=func)
            else:
                act(out=out_tile[:], in_=ps[:M, :Nf], func=func, bias=bias)
            return out_tile

        xT = xE[:, 0:256]
        eT = xE[:, 256:512]

        xpT = mm_to_sbuf(cv("node_proj"), xT, 128, 256, "xpT")
        epT = mm_to_sbuf(cv("emb_proj"), eT, 128, 256, "epT")
        HbT = mm_to_sbuf(cv("conv_w0"), epT[:], 128, 256, "HbT",
                         bias=cv("conv_b"), func=AF.Tanh,
                         extra=(cv("conv_w1"), xpT[:]))
        HsT = mm_to_sbuf(cv("conv_w0"), epT[:], 128, 256, "HsT",
                         bias=cv("conv_b"), func=AF.Tanh)
        HbsT = small.tile([128, 512], F32, tag="HbsT")
        nc.vector.tensor_copy(out=HbsT[:, 0:256], in_=HbT[:])
        nc.vector.tensor_copy(out=HbsT[:, 256:512], in_=HsT[:])
        MT = mm_to_sbuf(cv("lin2_w"), HbsT[:], 64, 512, "MT",
                        bias=cv("lin2_b"))
        PbT = mm_to_sbuf(cv("normal_proj"), MT[:, 0:256], 64, 256, "PbT")
        PsT = mm_to_sbuf(cv("masked_proj"), MT[:, 256:512], 64, 256, "PsT")
        PT = small.tile([64, 512], F32, tag="PT")
        nc.vector.tensor_copy(out=PT[:, 0:256], in_=PbT[:])
        nc.vector.tensor_copy(out=PT[:, 256:512], in_=PsT[:])
        XLcatT = mm_to_sbuf(cv("g1_wl"), PT[:], 128, 512, "XLcatT",
                            bias=cv("g1_bl"))
        XRcatT = mm_to_sbuf(cv("g1_wr"), PT[:], 128, 512, "XRcatT",
                            bias=cv("g1_br"))
        XLT, XLsT = XLcatT[:, 0:256], XLcatT[:, 256:512]
        XRT, XRsT = XRcatT[:, 0:256], XRcatT[:, 256:512]

        # store XL tables first (they gate the p1 gathers), XR right after
        xlcat_sb = small.tile([128, 4 * 128], BF16, tag="cat_xl")
        for k in range(4):
            ps = psum.tile([128, 128], F32, tag="ps")
            nc.tensor.transpose(ps[:], XLcatT[:, 128 * k:128 * (k + 1)],
                                ident[:])
            nc.vector.tensor_copy(out=xlcat_sb[:, 128 * k:128 * (k + 1)],
                                  in_=ps[:])
            if k < 2:
                sdma(out=D["XLbase"][128 * k:128 * (k + 1), :],
                     in_=xlcat_sb[:, 128 * k:128 * (k + 1)])
            dma(out=D["XLcat"][128 * k:128 * (k + 1), :],
                in_=xlcat_sb[:, 128 * k:128 * (k + 1)])
        xrcat_sb = small.tile([128, 4 * 128], BF16, tag="cat_xr")
        for k in range(4):
            ps = psum.tile([128, 128], F32, tag="ps")
            nc.tensor.transpose(ps[:], XRcatT[:, 128 * k:128 * (k + 1)],
                                ident[:])
            nc.vector.tensor_copy(out=xrcat_sb[:, 128 * k:128 * (k + 1)],
                                  in_=ps[:])
            dma(out=D["XRcat"][128 * k:128 * (k + 1), :],
                in_=xrcat_sb[:, 128 * k:128 * (k + 1)])

        # warmup: tiny gather preloads the gpsimd DMAGatherAnt ucode
        # (the first invocation otherwise pays ~5us of library load on the
        # critical path).  Reads uninitialized T1 bytes; result unused.
        warm = big.tile([128, 192], F32, tag="warm")
        dgather(warm[:].rearrange("p (k f) -> p k f", k=1),
                D["T1"][:], iv16("IDX1W0")[:, 0:8], 128, 192)

        # -------- big gathers on gpsimd (8.6ns/row desc-gen) --------
        xlg1 = []
        for h in range(2):
            xlg = big.tile([128, 4 * S1 * F], BF16, tag="p1_xlg%d" % h)
            half = 2 * S1 * F
            for j in range(2):
                dgather(xlg[:, j * half:(j + 1) * half]
                        .rearrange("p (k f) -> p k f", k=2 * S1),
                        D["XLbase"][:],
                        iv16("IDX1W%d" % h)[:, j * 2 * S1 * 8:
                                            (j + 1) * 2 * S1 * 8],
                        2 * S1 * 128, F)
            xlg1.append(xlg)
        xlgb = big.tile([128, SB * F], BF16, tag="b_xlg")
        dgather(xlgb[:].rearrange("p (k f) -> p k f", k=SB),
                D["XLcat"][:], iv16("IDX_BW"), SB * 128, F)
        t1xrg = big.tile([128, KA * F], BF16, tag="a_t1xr")
        dgather(t1xrg[:].rearrange("p (k f) -> p k f", k=KA),
                D["XRcat"][:], iv16("IDX_A_T1W"), KA * 128, F)
        # t1nd / xl2g / rare gathers are issued later (their tables are
        # written mid-kernel); gpsimd stalls there until writers finish.

        # -------- small row-selects on PE (one-hot matmuls) --------
        xrr1 = []
        for h in range(2):
            xrr = big.tile([128, 4, F], BF16, tag="p1_xrr%d" % h)
            for t in range(4):
                row_select([bv("OHXR1")[:, 128 * (4 * h + t):
                                        128 * (4 * h + t + 1)]],
                           [xrcat_sb[:, 128 * h:128 * (h + 1)]],
                           xrr[:, t, :], "xr1_%d_%d" % (h, t))
            xrr1.append(xrr)
        # ---------------- shared GAT edge stage ----------------
        def edge_stage(xlg_tile, nslot, mask_ap, att, xr_tile, tagp,
                       lr_act=False):
            """xlg_tile [128, nslot*F] gathered xl rows (consumed -> w*xl).
            xr_tile [128, F]; returns w tile [128, nslot, NH]."""
            xlg3 = xlg_tile[:].rearrange("p (s f) -> p s f", s=nslot)
            u = big.tile([128, nslot * F], BF16, tag=tagp + "_u")
            u3 = u[:].rearrange("p (s f) -> p s f", s=nslot)
            tt(out=u3, in0=xlg3,
               in1=xr_tile[:].rearrange("p f -> p () f")
               .to_broadcast([128, nslot, F]), op=OP.add)
            lrelu(u[:], nslot * F, tagp + "_u", dt=BF16, on_act=lr_act)
            attb = att.rearrange("p (h f) -> p () h f", h=NH) \
                .to_broadcast([128, nslot, NH, C2])
            u4 = u[:].rearrange("p (s h f) -> p s h f", s=nslot, h=NH)
            tt(out=u4, in0=u4, in1=attb, op=OP.mult)
            lg = small.tile([128, nslot, NH], F32, tag=tagp + "_lg")
            red(out=lg[:], in_=u4, axis=AX.X, op=OP.add)
            act(out=lg[:], in_=lg[:], func=AF.Exp)
            mb = mask_ap.rearrange("p s -> p s ()") \
                .to_broadcast([128, nslot, NH])
            tt(out=lg[:], in0=lg[:], in1=mb, op=OP.mult)
            wb = lg[:].rearrange("p s h -> p s h ()") \
                .to_broadcast([128, nslot, NH, C2])
            xlg4 = xlg_tile[:].rearrange("p (s h f) -> p s h f", s=nslot,
                                         h=NH)
            tt(out=xlg4, in0=xlg4, in1=wb, op=OP.mult)
            return lg

        def softmax_combine(xlg_tile, lg, nslot, tagp):
            comb = small.tile([128, F + NH], F32, tag=tagp + "_comb")
            red(out=comb[:, 0:F],
                in_=xlg_tile[:].rearrange("p (s f) -> p f s", s=nslot),
                axis=AX.X, op=OP.add)
            red(out=comb[:, F:F + NH],
                in_=lg[:].rearrange("p s h -> p h s"),
                axis=AX.X, op=OP.add)
            return comb

        # ---------------- phase 1: base GAT layer 1 ----------------
        att1 = bfcast("att1")
        att2 = bfcast("att2")
        cmb1bf = bfcast("CMB1")
        msk1 = bfcast("MSK1")
        mskbd = bfcast("MSKBD")
        mskd = bfcast("MSKD")
        g1b_chunks = []
        for h in range(2):
            xlg = xlg1[h]
            xrr = xrr1[h]
            tg = "p1h%d" % h
            u = big.tile([128, 4 * S1 * F], BF16, tag=tg + "_u")
            lr = big.tile([128, 4 * S1 * F], BF16, tag="lr_" + tg)
            half = 2 * S1 * F
            for j in range(2):
                uj = u[:, j * half:(j + 1) * half]
                xlgj = xlg[:, j * half:(j + 1) * half] \
                    .rearrange("p (t s f) -> p t s f", t=2, s=S1)
                tt(out=uj.rearrange("p (t s f) -> p t s f", t=2, s=S1),
                   in0=xlgj,
                   in1=xrr[:, 2 * j:2 * j + 2, :]
                   .rearrange("p t f -> p t () f")
                   .to_broadcast([128, 2, S1, F]), op=OP.add)
                lrj = lr[:, j * half:(j + 1) * half]
                act(out=lrj, in_=uj, func=AF.Identity, scale=NEG)
                tt(out=uj, in0=uj, in1=lrj, op=OP.max)
            attb = att1.rearrange("p (h f) -> p () () h f", h=NH) \
                .to_broadcast([128, 4, S1, NH, C2])
            u5 = u[:].rearrange("p (t s h f) -> p t s h f", t=4, s=S1,
                                h=NH)
            tt(out=u5, in0=u5, in1=attb, op=OP.mult)
            lg = small.tile([128, 4, S1, NH], F32, tag=tg + "_lg")
            red(out=lg[:], in_=u5, axis=AX.X, op=OP.add)
            act(out=lg[:], in_=lg[:], func=AF.Exp)
            msk = msk1[:, 4 * S1 * h:4 * S1 * (h + 1)] \
                .rearrange("p (t s) -> p t s", t=4)
            mb = msk.rearrange("p t s -> p t s ()") \
                .to_broadcast([128, 4, S1, NH])
            tt(out=lg[:], in0=lg[:], in1=mb, op=OP.mult)
            wb = lg[:].rearrange("p t s h -> p t s h ()") \
                .to_broadcast([128, 4, S1, NH, C2])
            xlg5 = xlg[:].rearrange("p (t s h f) -> p t s h f", t=4, s=S1,
                                    h=NH)
            tt(out=xlg5, in0=xlg5, in1=wb, op=OP.mult)

            # num: per-slot PE accumulation (replaces the strided DVE
            # reduce); den: small strided reduce + 4 tiny matmuls
            den = small.tile([128, 4, NH], F32, tag=tg + "_den")
            red(out=den[:], in_=lg[:].rearrange("p t s h -> p t h s"),
                axis=AX.X, op=OP.add)
            nd_ps = psum_acc.tile([128, F + NH], F32, tag=tg + "_ndps")
            for t in range(4):
                lh = cmb1bf[:, 128 * (4 * h + t):128 * (4 * h + t + 1)]
                for si in range(S1):
                    k = t * S1 + si
                    nc.tensor.matmul(nd_ps[:, 0:F], lh,
                                     xlg[:, k * F:(k + 1) * F],
                                     start=(t == 0 and si == 0),
                                     stop=(t == 3 and si == S1 - 1))
            cmb1 = cv("CMB1")
            for t in range(4):
                nc.tensor.matmul(nd_ps[:, F:F + NH],
                                 cmb1[:, 128 * (4 * h + t):
                                      128 * (4 * h + t + 1)],
                                 den[:, t, :],
                                 start=(t == 0), stop=(t == 3))
            nd = small.tile([128, F + NH], F32, tag=tg + "_nd")
            act(out=nd[:], in_=nd_ps[:], func=AF.Identity)
            sdma(out=D["T1"][128 * h:128 * (h + 1), 0:F + NH], in_=nd[:])
            g1b = head_mean_bias_elu(nd[:], 128, cv("g1bias"), tg + "_g")
            g1b_chunks.append(g1b)

        # remaining row-selects (issued after p1's lrelu ACT scales so
        # they don't hog the scalar queue ahead of them)
        xrrb = big.tile([128, F], BF16, tag="b_xrr")
        row_select([bv("OHB")[:, 0:128], bv("OHB")[:, 128:256]],
                   [xrcat_sb[:, 256:384], xrcat_sb[:, 384:512]],
                   xrrb[:], "xrb")
        xlv = big.tile([128, F], BF16, tag="a_xlv")
        row_select([bv("OHXLV")[:, 0:128], bv("OHXLV")[:, 128:256]],
                   [xlcat_sb[:, 0:128], xlcat_sb[:, 128:256]],
                   xlv[:], "xlv")
        xlsv = big.tile([128, F], BF16, tag="a_xlsv")
        row_select([bv("OHXLSV")[:, 0:128], bv("OHXLSV")[:, 128:256]],
                   [xlcat_sb[:, 256:384], xlcat_sb[:, 384:512]],
                   xlsv[:], "xlsv")

        # g1_base^T -> XL2_base (T2 rows 0:256)
        g1bT = small.tile([64, 256], F32, tag="g1bT")
        for h in range(2):
            ps = psum.tile([64, 128], F32, tag="ps")
            nc.tensor.transpose(ps[:], g1b_chunks[h], ident[:])
            act(out=g1bT[:, 128 * h:128 * (h + 1)], in_=ps[:],
                func=AF.Identity)
        for h in range(2):
            ps = psum.tile([128, 128], F32, tag="ps")
            nc.tensor.matmul(ps[:], g1bT[:, 128 * h:128 * (h + 1)],
                             cv("g2_wl"), start=True, stop=True)
            sb = small.tile([128, 128], BF16, tag="p15_sb%d" % h)
            act(out=sb[:], in_=ps[:], func=AF.Identity)
            sdma(out=D["T2"][128 * h:128 * (h + 1), :], in_=sb[:])

        # ---------------- (b): full recompute of dst v ----------------
        lgb = edge_stage(xlgb, SB, mskbd[:], att1, xrrb, "b")
        combb = softmax_combine(xlgb, lgb, SB, "b")
        ndb_ps = psum.tile([VPC, F + NH], F32, tag="ps")
        nc.tensor.matmul(ndb_ps[:], cv("CMBBD"), combb[:],
                         start=True, stop=True)
        ndb = small.tile([VPC, F + NH], F32, tag="b_nd")
        act(out=ndb[:], in_=ndb_ps[:], func=AF.Identity)
        g1self = head_mean_bias_elu(ndb[:], VPC, cv("g1bias"), "bg1")

        ps_t = psum.tile([C2, VPC], F32, tag="ps")
        nc.tensor.transpose(ps_t[:], g1self, ident[:VPC, :VPC])
        g1sT = small.tile([C2, VPC], F32, tag="g1sT")
        act(out=g1sT[:], in_=ps_t[:], func=AF.Identity)
        ps_l = psum.tile([VPC, F], F32, tag="ps")
        nc.tensor.matmul(ps_l[:], g1sT[:], cv("g2_wl"), start=True, stop=True)
        sb_l = small.tile([VPC, F], BF16, tag="b_sbl")
        act(out=sb_l[:], in_=ps_l[:], func=AF.Identity)
        ps_r = psum.tile([VPC, F], F32, tag="ps")
        nc.tensor.matmul(ps_r[:], g1sT[:], cv("g2_wr"), start=True, stop=True)
        sb_r = small.tile([VPC, F], BF16, tag="b_sbr")
        tt(out=sb_r[:], in0=ps_r[:], in1=cv("blr")[:VPC, :], op=OP.add)
        # sb_l / sb_r stay in SBUF: the d-phase self slots and xr rows are
        # served straight from them (no T2-self / XR2S roundtrip).

        # -------- late gathers (gpsimd order: t1nd -> xl2g) --------
        t1nd = big.tile([128, KA * 192], F32, tag="a_t1nd")
        dgather(t1nd[:].rearrange("p (k f) -> p k f", k=KA),
                D["T1"][:], iv16("IDX_A_T1W"), KA * 128, 192)
        xl2g = big.tile([128, SB * F], BF16, tag="d_xlg")
        sb_half = SB // 2
        for j, (k0, k1) in enumerate(((0, sb_half), (sb_half, SB))):
            dgather(xl2g[:, k0 * F:k1 * F]
                    .rearrange("p (k f) -> p k f", k=k1 - k0),
                    D["T2"][:], iv16("IDX_DW")[:, k0 * 8:k1 * 8],
                    (k1 - k0) * 128, F)
        # d/rare xr rows straight from b's sb_r SBUF tile via PE
        xr2r = big.tile([128, F], BF16, tag="d_xrr")
        row_select([bv("OHD2")[:32, :]], [sb_r[:]], xr2r[:], "xr2")
        xr2rare = big.tile([128, F], BF16, tag="r_xr2")
        row_select([bv("OHRARE2")[:32, :]], [sb_r[:]], xr2rare[:], "xr2r")

        # ---------------- (a): light dst updates ----------------
        t1g3 = t1nd[:].rearrange("p (k f) -> p k f", k=KA)  # f = 192
        t1xr = t1xrg[:].rearrange("p (k f) -> p k f", k=KA)
        t1num = t1g3[:, :, 0:F]
        t1den = t1g3[:, :, F:F + NH]
        xlv3 = xlv[:].rearrange("p f -> p () f").to_broadcast([128, KA, F])
        xlsv3 = xlsv[:].rearrange("p f -> p () f").to_broadcast([128, KA, F])
        ca = cv("C_A")

        def logits_expC(xl3, tg):
            # f32: wn - wo is a cancellation-sensitive delta
            u = big.tile([128, KA * F], F32, tag="a_u" + tg)
            u3 = u[:].rearrange("p (k f) -> p k f", k=KA)
            tt(out=u3, in0=xl3, in1=t1xr, op=OP.add)
            lrelu(u[:], KA * F, "a_u" + tg, on_act=True)
            attb = cv("att1").rearrange("p (h f) -> p () h f", h=NH) \
                .to_broadcast([128, KA, NH, C2])
            u4 = u[:].rearrange("p (k h f) -> p k h f", k=KA, h=NH)
            tt(out=u4, in0=u4, in1=attb, op=OP.mult)
            lw = small.tile([128, KA, NH], F32, tag="a_lw" + tg)
            red(out=lw[:], in_=u4, axis=AX.X, op=OP.add)
            act(out=lw[:], in_=lw[:], func=AF.Exp)
            cb = ca.rearrange("p k -> p k ()").to_broadcast([128, KA, NH])
            tt(out=lw[:], in0=lw[:], in1=cb, op=OP.mult)
            return lw

        wn = logits_expC(xlsv3, "n")    # C * w_new
        wo = logits_expC(xlv3, "o")     # C * w_old

        # ---- (d) main edge stage early: does not need (a)'s output ----
        lgd = edge_stage(xl2g, SB, mskd[:], att2, xr2r, "d")
        combd = softmax_combine(xl2g, lgd, SB, "d")

        # ---- (a) tail: nd deltas -> G1L ----
        dden = small.tile([128, KA, NH], F32, tag="a_dden")
        tt(out=dden[:], in0=wn[:], in1=wo[:], op=OP.subtract)
        tt(out=dden[:], in0=dden[:], in1=t1den, op=OP.add)
        dnum = big.tile([128, KA * F], F32, tag="a_dnum")
        dnum4 = dnum[:].rearrange("p (k h f) -> p k h f", k=KA, h=NH)
        dnum3 = dnum[:].rearrange("p (k f) -> p k f", k=KA)
        tmp = big.tile([128, KA * F], F32, tag="a_tmp")
        tmp4 = tmp[:].rearrange("p (k h f) -> p k h f", k=KA, h=NH)
        tmp3 = tmp[:].rearrange("p (k f) -> p k f", k=KA)
        wnb = wn[:].rearrange("p k h -> p k h ()") \
            .to_broadcast([128, KA, NH, C2])
        xlsv4 = xlsv[:].rearrange("p (h f) -> p () h f", h=NH) \
            .to_broadcast([128, KA, NH, C2])
        tt(out=dnum4, in0=xlsv4, in1=wnb, op=OP.mult)
        wob = wo[:].rearrange("p k h -> p k h ()") \
            .to_broadcast([128, KA, NH, C2])
        xlv4 = xlv[:].rearrange("p (h f) -> p () h f", h=NH) \
            .to_broadcast([128, KA, NH, C2])
        tt(out=tmp4, in0=xlv4, in1=wob, op=OP.mult)
        tt(out=dnum3, in0=dnum3, in1=tmp3, op=OP.subtract)
        tt(out=dnum3, in0=dnum3, in1=t1num, op=OP.add)
        nc.vector.reciprocal(out=dden[:], in_=dden[:])
        ddb = dden[:].rearrange("p k h -> p k h ()") \
            .to_broadcast([128, KA, NH, C2])
        tt(out=dnum4, in0=dnum4, in1=ddb, op=OP.mult)
        radd = big.tile([128, KA, C2], F32, tag="a_radd")
        tt(out=radd[:], in0=dnum4[:, :, 0, :], in1=dnum4[:, :, 1, :],
           op=OP.add)
        ts_mul(radd[:], radd[:], 0.5)
        g1bb = cv("g1bias").rearrange("p f -> p () f").to_broadcast(
            [128, KA, C2])
        tt(out=radd[:], in0=radd[:], in1=g1bb, op=OP.add)
        radd_flat = radd[:].rearrange("p k f -> p (k f)")
        elu_inplace(radd_flat, big, KA * C2, "a_elu")

        # ---------------- rare mini edge-stage ----------------
        # select the rare (v,d) g1 rows from radd across partitions via PE
        grare_ps = psum.tile([128, C2], F32, tag="ps")
        ohk = cv("OHRARE_K")
        for k in range(KA):
            nc.tensor.matmul(grare_ps[:], ohk[:, 128 * k:128 * (k + 1)],
                             radd[:, k, :], start=(k == 0),
                             stop=(k == KA - 1))
        grare = small.tile([128, C2], F32, tag="r_g")
        act(out=grare[:], in_=grare_ps[:], func=AF.Identity)
        ps_rt = psum.tile([C2, 128], F32, tag="ps")
        nc.tensor.transpose(ps_rt[:], grare[:], ident[:])
        grT = small.tile([C2, 128], F32, tag="grT")
        nc.vector.tensor_copy(out=grT[:], in_=ps_rt[:])
        ps_rm = psum.tile([128, F], F32, tag="ps")
        nc.tensor.matmul(ps_rm[:], grT[:], cv("g2_wl"), start=True, stop=True)
        xl2rare = small.tile([128, F], F32, tag="r_xl2")
        nc.vector.tensor_copy(out=xl2rare[:], in_=ps_rm[:])
        # logits for the rare (v, d) pairs
        ur = small.tile([128, F], F32, tag="r_u")
        tt(out=ur[:], in0=xl2rare[:], in1=xr2rare[:], op=OP.add)
        ur2 = small.tile([128, F], F32, tag="r_u2")
        ts_mul(ur2[:], ur[:], NEG)
        tt(out=ur[:], in0=ur[:], in1=ur2[:], op=OP.max)
        tt(out=ur[:], in0=ur[:], in1=cv("att2"), op=OP.mult)
        lgr = small.tile([128, NH], F32, tag="r_lg")
        red(out=lgr[:], in_=ur[:].rearrange("p (h f) -> p h f", h=NH),
            axis=AX.X, op=OP.add)
        act(out=lgr[:], in_=lgr[:], func=AF.Exp)
        tt(out=lgr[:], in0=lgr[:],
           in1=cv("C_RARE").to_broadcast([128, NH]), op=OP.mult)
        rare_rhs = small.tile([128, F + NH], F32, tag="r_rhs")
        lgrb = lgr[:].rearrange("p h -> p h ()").to_broadcast([128, NH, C2])
        tt(out=rare_rhs[:, 0:F].rearrange("p (h f) -> p h f", h=NH),
           in0=xl2rare[:].rearrange("p (h f) -> p h f", h=NH),
           in1=lgrb, op=OP.mult)
        nc.vector.tensor_copy(out=rare_rhs[:, F:F + NH], in_=lgr[:])

        # ---------------- self mini stage (one self loop per dst) -------
        us = small.tile([VPC, F], F32, tag="s_u")
        tt(out=us[:], in0=sb_l[:], in1=sb_r[:], op=OP.add)
        us2 = small.tile([VPC, F], F32, tag="s_u2")
        ts_mul(us2[:], us[:], NEG)
        tt(out=us[:], in0=us[:], in1=us2[:], op=OP.max)
        tt(out=us[:], in0=us[:], in1=cv("att2")[:VPC, :], op=OP.mult)
        lgs = small.tile([VPC, NH], F32, tag="s_lg")
        red(out=lgs[:], in_=us[:].rearrange("p (h f) -> p h f", h=NH),
            axis=AX.X, op=OP.add)
        act(out=lgs[:], in_=lgs[:], func=AF.Exp)
        tt(out=lgs[:], in0=lgs[:],
           in1=cv("C_SELF")[:VPC, :].to_broadcast([VPC, NH]), op=OP.mult)
        self_rhs = small.tile([VPC, F + NH], F32, tag="s_rhs")
        lgsb = lgs[:].rearrange("p h -> p h ()").to_broadcast([VPC, NH, C2])
        tt(out=self_rhs[:, 0:F].rearrange("p (h f) -> p h f", h=NH),
           in0=sb_l[:].rearrange("p (h f) -> p h f", h=NH),
           in1=lgsb, op=OP.mult)
        nc.vector.tensor_copy(out=self_rhs[:, F:F + NH], in_=lgs[:])

        # ------- (d) combine: main + rare + self into one PSUM ----------
        ndd_ps = psum.tile([VPC, F + NH], F32, tag="ps")
        nc.tensor.matmul(ndd_ps[:], cv("CMBBD"), combd[:],
                         start=True, stop=False)
        nc.tensor.matmul(ndd_ps[:], cv("OH_RARE"), rare_rhs[:],
                         start=False, stop=False)
        nc.tensor.matmul(ndd_ps[:], ident[:VPC, :VPC], self_rhs[:],
                         start=False, stop=True)
        ndd = small.tile([VPC, F + NH], F32, tag="d_nd")
        act(out=ndd[:], in_=ndd_ps[:], func=AF.Identity)
        g2row = head_mean_bias_elu(ndd[:], VPC, cv("g2bias"), "dg2")

        # out = tanh(g2row @ rec_w + rec_b)
        ps_ot = psum.tile([C2, VPC], F32, tag="ps")
        nc.tensor.transpose(ps_ot[:], g2row, ident[:VPC, :VPC])
        g2T = small.tile([C2, VPC], F32, tag="g2T")
        act(out=g2T[:], in_=ps_ot[:], func=AF.Identity)
        ps_om = psum.tile([C2, VPC], F32, tag="ps")
        nc.tensor.matmul(ps_om[:], cv("rec_w"), g2T[:], start=True,
                         stop=True)
        outT = small.tile([C2, VPC], F32, tag="outT")
        act(out=outT[:], in_=ps_om[:], func=AF.Tanh, bias=cv("rec_b"))
        ps_of = psum.tile([VPC, C2], F32, tag="ps")
        nc.tensor.transpose(ps_of[:], outT[:], ident[:C2, :C2])
        outsb = small.tile([VPC, C2], F32, tag="outsb")
        act(out=outsb[:], in_=ps_of[:], func=AF.Identity)
        dma(out=D["out"][:], in_=outsb[:])


# --------------------------------------------------------------------------
# Entry point
# --------------------------------------------------------------------------

def _make_in_maps(inputs, shared, percore, dims):
    import ml_dtypes
    f32 = np.float32
    (off_f, nf), (off_i16, ni16), (off_bf, nbf) = _pack_layout(dims)

    def rep(v):
        a = np.asarray(v, f32).reshape(1, -1)
        return np.ascontiguousarray(np.broadcast_to(a, (128, a.shape[1])))

    f64 = np.float64
    lin2_w = np.asarray(inputs["lin2_w"], f64)
    lin2_b = np.asarray(inputs["lin2_b"], f64)
    nproj = np.asarray(inputs["normal_proj"], f64)
    mproj = np.asarray(inputs["masked_proj"], f64)
    g1_wl = np.asarray(inputs["g1_wl"], f64)
    g1_wr = np.asarray(inputs["g1_wr"], f64)
    g1_bl = np.asarray(inputs["g1_bl"], f64)
    g1_br = np.asarray(inputs["g1_br"], f64)
    vals = {
        "conv_b": np.asarray(inputs["conv_b"], f32).reshape(128, 1),
        "WLn": (lin2_w @ nproj @ g1_wl).astype(f32),
        "WLm": (lin2_w @ mproj @ g1_wl).astype(f32),
        "WRn": (lin2_w @ nproj @ g1_wr).astype(f32),
        "WRm": (lin2_w @ mproj @ g1_wr).astype(f32),
        "bLn": (lin2_b @ nproj @ g1_wl + g1_bl).astype(f32).reshape(128, 1),
        "bLm": (lin2_b @ mproj @ g1_wl + g1_bl).astype(f32).reshape(128, 1),
        "bRn": (lin2_b @ nproj @ g1_wr + g1_br).astype(f32).reshape(128, 1),
        "bRm": (lin2_b @ mproj @ g1_wr + g1_br).astype(f32).reshape(128, 1),
        "rec_b": np.asarray(inputs["rec_b"], f32).reshape(64, 1),
        "att1": rep(inputs["g1_att"]),
        "att2": rep(inputs["g2_att"]),
        "g1bias": rep(inputs["g1_bias"]),
        "g2bias": rep(inputs["g2_bias"]),
        "blr": rep(inputs["g2_bl"] + inputs["g2_br"]),
        "CMB1": shared["CMB1"].transpose(1, 0, 2, 3).reshape(128, -1),
        "MSK1": shared["MSK1"].transpose(1, 0, 2).reshape(128, -1),
    }
    for nm in ("node_proj", "emb_proj", "conv_w0", "conv_w1", "g2_wl",
               "g2_wr", "rec_w"):
        vals[nm] = np.asarray(inputs[nm], f32)

    x = np.asarray(inputs["x"], f32)
    E = np.asarray(inputs["E_emb"], f32)
    xE = np.concatenate([x.T, E.T], axis=1)   # [64, 512]

    def fill(off_map, total, npdtype, core_vals):
        out = np.zeros((128, total), npdtype)
        for nm, (o, rows, cols) in off_map.items():
            a = core_vals[nm]
            assert a.shape[1] == cols and a.shape[0] <= rows, \
                (nm, a.shape, rows, cols)
            out[:a.shape[0], o:o + cols] = a
        return out

    in_maps = []
    for c in range(NCORES):
        t = percore[c]
        cvals = dict(vals)
        for nm in ("CMBBD", "OH_RARE", "C_RARE", "C_SELF", "MSKBD",
                   "MSKD", "C_A"):
            cvals[nm] = t[nm]
        cvals["OHRARE_K"] = t["OHRARE_K"].transpose(1, 0, 2) \
            .reshape(128, -1)
        i16vals = {"IDX1W0": shared["IDX1W"][0],
                   "IDX1W1": shared["IDX1W"][1],
                   "IDX_BW": t["IDX_BW"], "IDX_DW": t["IDX_DW"],
                   "IDX_A_T1W": t["IDX_A_T1W"]}
        bfvals = {"OHXR1": shared["OHXR1"].transpose(2, 0, 1, 3)
                  .reshape(128, 8 * 128),
                  "OHB": t["OHB"].transpose(1, 0, 2).reshape(128, 256),
                  "OHXLV": t["OHXLV"].transpose(1, 0, 2).reshape(128, 256),
                  "OHXLSV": t["OHXLSV"].transpose(1, 0, 2)
                  .reshape(128, 256),
                  "OHD2": t["OHD2"], "OHRARE2": t["OHRARE2"]}
        in_maps.append({
            "packf": fill(off_f, nf, np.float32, cvals),
            "packi16": fill(off_i16, ni16, np.int16, i16vals),
            "packbf": fill(off_bf, nbf, ml_dtypes.bfloat16, bfvals),
            "xE": np.ascontiguousarray(xE),
        })
    return in_maps


_CACHE = {}
TRACE = False          # set by test.py to capture NTFF profiles
LAST_RESULT = None


def kernel(**inputs):
    global LAST_RESULT
    inputs = {k: np.asarray(v) for k, v in inputs.items()}
    shared, percore, dims = _build_tables(inputs["edge_index"])
    key = (dims["S1"], dims["SB"], dims["KA"])
    if key not in _CACHE:
        _CACHE[key] = _build_program(dims)
    nc = _CACHE[key]
    in_maps = _make_in_maps(inputs, shared, percore, dims)
    kw = {}
    if TRACE:
        kw = dict(trace=True, trace_cores=list(range(NCORES)))
    res = run_bass_kernel_spmd(nc, in_maps, core_ids=list(range(NCORES)),
                               **kw)
    LAST_RESULT = res
    out = np.concatenate([res.results[c]["out"] for c in range(NCORES)],
                         axis=0)
    return out.astype(np.float32)
